# revision 19
# baseline (speedup 1.0000x reference)
"""DualMaskRoIPool Trainium2 kernel (raw Bass, bf16 band).

Strategy: shard the 64 ROIs across 8 NeuronCores clustered by union-box row
range; each core DMAs only its row slice of the feature map, converted to
bf16 on the host (the 2e-2 rel-err budget dwarfs bf16 rounding).  ROI
coordinates are known when `kernel()` runs, so each core gets a specialized
straight-line raw-Bass program (no Tile framework -> no per-instruction
semaphore overhead):

  per ROI: the union-box window is materialized in a dedicated SBUF val
  buffer by one prep engine (ACT / GPSIMD / DVE, balanced by a calibrated
  cost model): complement rectangles memset to 0, mask rectangles copied
  from the band.  DVE then max-reduces the adaptive 7x7 bin grid (one- or
  two-stage, with all-zero bins skipped via tmp memsets).  Cross-engine
  sync is one semaphore wait per ROI, embedded in the first consumer
  instruction.

The 8 per-core programs are dispatched concurrently via the bass2jax PJRT
path.
"""

import numpy as np
import ml_dtypes

PH, PW = 7, 7
SCALE = 0.0625
C, H, W = 128, 56, 56
NCORES = 8
NROIS = 64
NEG = -1e30

# calibrated TRN2 per-instruction costs (ns)
DVE_FIX = 92.0
DVE_EL_RED = 1.04     # tensor_reduce, any dtype
DVE_EL_CP = 0.26      # tensor_copy bf16 (4x_2p)
ACT_FIX = 265.0
ACT_EL = 0.83
GPS_FIX = 155.0
GPS_EL_MS = 0.83      # memset
GPS_EL_CP = 1.39      # tensor_copy (0.6 eff)
ROW_NS = 45.0         # marginal DMA ns per extra band row (bf16)
RAMP_NS = 40.0        # serial wait per row of the lead ROI's window


# ----------------------------------------------------------------- geometry

def _zoom(rois):
    s = np.round(rois[:, 1:].astype(np.float32) * np.float32(SCALE)).astype(np.int32)
    x1 = np.where(s[:, 0] >= W, W - 1, s[:, 0])
    y1 = np.where(s[:, 1] >= H, H - 1, s[:, 1])
    x2 = np.where(s[:, 2] >= W, W - 1, s[:, 2])
    y2 = np.where(s[:, 3] >= H, H - 1, s[:, 3])
    return x1, y1, x2, y2


def _bin_edges(lo, extent):
    starts = np.array([lo + (i * extent) // PH for i in range(PH)], np.int64)
    ends = np.array([lo + ((i + 1) * extent + PH - 1) // PH for i in range(PH)], np.int64)
    return starts, ends - starts


def _runs_idx(idxs, starts, lens):
    runs = []
    i = 0
    n = len(idxs)
    while i < n:
        cnt = 1
        gap = 1
        while i + cnt < n:
            if idxs[i + cnt] != idxs[i + cnt - 1] + 1:
                break
            g = int(starts[idxs[i + cnt]] - starts[idxs[i + cnt - 1]])
            if lens[idxs[i + cnt]] != lens[idxs[i]]:
                break
            if cnt == 1:
                gap = g
            elif g != gap:
                break
            cnt += 1
        runs.append((idxs[i], cnt, gap, int(lens[idxs[i]])))
        i += cnt
    return runs


def _runs(starts, lens):
    return _runs_idx(list(range(PH)), starts, lens)


def _complement_rects(mask):
    h, w = mask.shape
    rects = []
    r = 0
    while r < h:
        r2 = r
        while r2 + 1 < h and np.array_equal(mask[r2 + 1], mask[r]):
            r2 += 1
        row = mask[r]
        x = 0
        while x < w:
            if not row[x]:
                x2 = x
                while x2 + 1 < w and not row[x2 + 1]:
                    x2 += 1
                rects.append((r, r2 + 1, x, x2 + 1))
                x = x2 + 1
            else:
                x += 1
        r = r2 + 1
    return rects


def _plan_roi(mask, rs, hgt, cs, wdt):
    """Choose one- vs two-stage reduce plan with calibrated DVE costs."""
    h, w = mask.shape
    iruns = _runs(rs, hgt)
    jruns = _runs(cs, wdt)
    sj = sum(nj * wd for (_, nj, _, wd) in jruns)
    si = sum(ni * hg for (_, ni, _, hg) in iruns)

    one_cost = si * sj * DVE_EL_RED + DVE_FIX * len(iruns) * len(jruns)

    nzrow = mask.any(axis=1)
    segments = []
    r = 0
    while r < h:
        if nzrow[r]:
            r2 = r
            while r2 + 1 < h and nzrow[r2 + 1]:
                r2 += 1
            segments.append((r, r2 + 1))
            r = r2 + 1
        else:
            r += 1
    nzero_rows = h - int(nzrow.sum())
    if len(segments) > 1 and nzero_rows * sj * DVE_EL_RED < \
            (len(segments) - 1) * len(jruns) * DVE_FIX:
        segments = [(0, h)]
    if not segments:
        segments = [(0, h)]

    xinstrs = []
    tmemsets = []   # (elem_offset, [dims]) zero-fills in tmp
    xcells = 0
    nxi = 0
    prev_end = 0
    for (a, b) in segments:
        if a > prev_end:
            tmemsets.append((prev_end * PW, [[1, (a - prev_end) * PW]]))
        prev_end = b
        seg_zero_col = ~mask[a:b].any(axis=0)
        kept = []
        for j in range(PH):
            c0 = int(cs[j])
            wd = int(wdt[j])
            if seg_zero_col[c0:c0 + wd].all() and \
                    (b - a) * wd * DVE_EL_RED > GPS_FIX + (b - a) * GPS_EL_MS:
                tmemsets.append((a * PW + j, [[PW, b - a]]))
            else:
                kept.append(j)
        runs = _runs_idx(kept, cs, wdt)
        xinstrs.append((a, b - a, runs))
        nxi += len(runs)
        xcells += (b - a) * sum(nj * wd for (_, nj, _, wd) in runs)

    if prev_end < h:
        tmemsets.append((prev_end * PW, [[1, (h - prev_end) * PW]]))

    two_cost = (xcells + PW * si) * DVE_EL_RED + DVE_FIX * (nxi + len(iruns))

    one = one_cost <= two_cost
    dve = min(one_cost, two_cost)
    return dict(one_stage=one, iruns=iruns, jruns=jruns,
                xinstrs=xinstrs, tmemsets=tmemsets, dve_ns=dve)


def _geometry(rois_1, rois_2):
    x1a, y1a, x2a, y2a = _zoom(np.asarray(rois_1))
    x1b, y1b, x2b, y2b = _zoom(np.asarray(rois_2))
    ux1 = np.minimum(x1a, x1b)
    uy1 = np.minimum(y1a, y1b)
    ux2 = np.maximum(x2a, x2b)
    uy2 = np.maximum(y2a, y2b)
    geoms = []
    for b in range(len(ux1)):
        lo_y, hi_y = int(uy1[b]), int(uy2[b])
        lo_x, hi_x = int(ux1[b]), int(ux2[b])
        h = hi_y - lo_y + 1
        w = hi_x - lo_x + 1
        mask = np.zeros((h, w), bool)
        mask[y1a[b] - lo_y:y2a[b] - lo_y + 1, x1a[b] - lo_x:x2a[b] - lo_x + 1] = True
        mask[y1b[b] - lo_y:y2b[b] - lo_y + 1, x1b[b] - lo_x:x2b[b] - lo_x + 1] = True
        rs, hgt = _bin_edges(0, h)      # window-relative
        cs, wdt = _bin_edges(0, w)
        rects = _complement_rects(mask)       # zero (complement) rects
        mrects = _complement_rects(~mask)     # mask rects to copy
        g = dict(uy1=lo_y, uy2=hi_y, ux1=lo_x, ux2=hi_x, h=h, w=w,
                 rects=rects, mrects=mrects, rs=rs, cs=cs, hgt=hgt, wdt=wdt)
        g.update(_plan_roi(mask, rs, hgt, cs, wdt))
        # prep work: memset complement + copy mask rects (disjoint cells)
        marea = sum((r1 - r0) * (c1 - c0) for r0, r1, c0, c1 in mrects)
        carea = sum((r1 - r0) * (c1 - c0) for r0, r1, c0, c1 in rects)
        g["marea"] = marea
        g["carea"] = carea
        g["prep_act"] = (len(mrects) + len(rects)) * ACT_FIX + marea * ACT_EL \
            + carea * ACT_EL + 1e9 * (len(rects) > 0)  # ACT can't memset safely
        g["prep_gps"] = (len(mrects) + len(rects)) * GPS_FIX \
            + marea * GPS_EL_CP + carea * GPS_EL_MS
        g["prep_dve"] = (len(mrects) + len(rects)) * DVE_FIX \
            + marea * DVE_EL_CP + carea * 0.52
        g["prep_act_copy"] = len(mrects) * ACT_FIX + marea * ACT_EL
        g["prep_gps_ms"] = len(rects) * GPS_FIX + carea * GPS_EL_MS
        g["cost"] = g["dve_ns"] + 250.0
        geoms.append(g)
    return geoms


# ---------------------------------------------------------------- balancing

def _partition_balanced(geoms):
    order = sorted(range(NROIS), key=lambda b: geoms[b]["uy1"] + geoms[b]["uy2"])
    costs = [geoms[b]["cost"] for b in order]
    pre = np.concatenate([[0], np.cumsum(costs)])
    n = NROIS
    lo = np.array([geoms[b]["uy1"] for b in order])
    hi = np.array([geoms[b]["uy2"] for b in order])

    def group_cost(i, j):
        span = hi[i:j].max() - lo[i:j].min() + 1
        return pre[j] - pre[i] + ROW_NS * span

    INF = float("inf")
    dp = np.full((NCORES + 1, n + 1), INF)
    cut = np.zeros((NCORES + 1, n + 1), np.int64)
    dp[0, 0] = 0.0
    for gidx in range(1, NCORES + 1):
        for j in range(gidx, n + 1):
            best, barg = INF, gidx - 1
            for i in range(gidx - 1, j):
                v = max(dp[gidx - 1, i], group_cost(i, j))
                if v < best:
                    best, barg = v, i
            dp[gidx, j] = best
            cut[gidx, j] = barg
    cuts = [n]
    j = n
    for gidx in range(NCORES, 0, -1):
        j = int(cut[gidx, j])
        cuts.append(j)
    cuts = cuts[::-1]
    groups = [list(order[cuts[c]:cuts[c + 1]]) for c in range(NCORES)]

    def gcost(ids):
        if not ids:
            return 0.0
        span = max(geoms[b]["uy2"] for b in ids) - min(geoms[b]["uy1"] for b in ids) + 1
        first = min(ids, key=lambda b: geoms[b]["uy2"])
        ramp = geoms[first]["uy2"] - geoms[first]["uy1"] + 1
        return sum(geoms[b]["cost"] for b in ids) + ROW_NS * span + RAMP_NS * ramp

    for _ in range(200):
        cc = [gcost(g) for g in groups]
        wi = int(np.argmax(cc))
        best = (0.0, None)
        for b in groups[wi]:
            for vi in range(NCORES):
                if vi == wi:
                    continue
                if len(groups[wi]) <= 1:
                    break
                nw = gcost([x for x in groups[wi] if x != b])
                nv = gcost(groups[vi] + [b])
                gain = cc[wi] - max(nw, nv, *(cc[t] for t in range(NCORES)
                                              if t not in (wi, vi)))
                if gain > best[0] + 1e-9:
                    best = (gain, ("m", b, vi))
                for b2 in groups[vi]:
                    nw = gcost([x for x in groups[wi] if x != b] + [b2])
                    nv = gcost([x for x in groups[vi] if x != b2] + [b])
                    gain = cc[wi] - max(nw, nv, *(cc[t] for t in range(NCORES)
                                                  if t not in (wi, vi)))
                    if gain > best[0] + 1e-9:
                        best = (gain, ("s", b, b2, vi))
        if best[1] is None:
            break
        if best[1][0] == "m":
            _, b, vi = best[1]
            groups[wi].remove(b)
            groups[vi].append(b)
        else:
            _, b, b2, vi = best[1]
            groups[wi].remove(b)
            groups[vi].remove(b2)
            groups[wi].append(b2)
            groups[vi].append(b)
    return groups


# ------------------------------------------------------------ program build

def _chunks_for(geoms, ylo, nrows):
    """Row chunks (rel. ylo): chunk0 covers the lead ROI, then thirds."""
    n = len(geoms)
    w1 = min(nrows, geoms[0]["uy2"] - ylo + 1)
    cuts = {w1, nrows}
    for t in ((n + 2) // 3, (2 * n + 2) // 3):
        if 0 < t < n:
            cuts.add(min(nrows, max(w1, geoms[t]["uy2"] - ylo + 1)))
    bounds = sorted(cuts)
    chunks = [(0, bounds[0])]
    chunks += [(a, b) for a, b in zip(bounds[:-1], bounds[1:]) if b > a]
    return chunks


DRAIN_NS = 300.0


def _assign_prep(geoms):
    """Per-ROI (copy_engine, x_engine) assignment over {'dve','gps'}.

    All complement/tmp memsets run up-front on GPS (hidden under the input
    DMA).  Copies go DVE-inline (cheap bf16 4x copies, zero sync) unless GPS
    relieves the DVE critical path; x-stages of two-stage ROIs can move to
    GPS as tensor_tensor max chains (requires GPS copies for that ROI)."""
    assign = []
    dve = 0.0
    for g in geoms:
        g["dve_prep"] = len(g["mrects"]) * DVE_FIX + g["marea"] * DVE_EL_CP
        g["gps_prep"] = len(g["mrects"]) * GPS_FIX + g["marea"] * GPS_EL_CP \
            + DRAIN_NS
        if not g["one_stage"]:
            dx = dy = 0.0
            gx = 0.0
            for (a, nrow, runs) in g["xinstrs"]:
                for (j0, nj, gj, wd) in runs:
                    dx += DVE_FIX + nrow * nj * wd * DVE_EL_RED
                    gx += wd * GPS_FIX + wd * nrow * nj * GPS_EL_CP
            for (i0, ni, gi, hgt) in g["iruns"]:
                dy += DVE_FIX + ni * PW * hgt * DVE_EL_RED
            g["dve_x"], g["dve_y"], g["gps_x"] = dx, dy, gx + DRAIN_NS
        else:
            g["dve_x"] = g["dve_ns"]
            g["dve_y"] = 0.0
            g["gps_x"] = None
        dve += g["dve_x"] + g["dve_y"]
    gload = sum(g["carea"] * GPS_EL_MS + len(g["rects"]) * GPS_FIX
                for g in geoms) * 0.3   # memsets mostly hidden under DMA
    dload = dve
    for g in geoms:
        # candidate plans: (cost-delta-dve, cost-delta-gps, tag)
        # copies stay DVE-inline: the tight GPS->DVE per-ROI handoff raced
        # on hardware (GPSIMD completion signal outruns its SBUF writes).
        assign.append(("dve", "dve"))
    return assign


def _build_core_program(geoms, ylo, nrows):
    import concourse.bacc as bacc
    import concourse.bass as bass
    from concourse import mybir

    f32 = mybir.dt.float32
    bf16 = mybir.dt.bfloat16
    nroi = len(geoms)
    nc = bacc.Bacc("TRN2", target_bir_lowering=False, debug=False)

    chunks = _chunks_for(geoms, ylo, nrows)
    nchunk = len(chunks)
    feat_ds = [nc.dram_tensor(f"feat{ci}", [C, (r1 - r0) * W], bf16,
                              kind="ExternalInput")
               for ci, (r0, r1) in enumerate(chunks)]
    out_d = nc.dram_tensor("out", [C, nroi * PH * PW], f32, kind="ExternalOutput")

    # --- SBUF layout (dedicated, no reuse) ---
    band = nc.alloc_sbuf_tensor("band", [C, nrows * W], bf16)
    vals = []
    tmps = []
    for k, g in enumerate(geoms):
        vals.append(nc.alloc_sbuf_tensor(f"val{k}", [C, g["h"] * g["w"]], bf16))
        if not g["one_stage"]:
            tmps.append(nc.alloc_sbuf_tensor(f"tmp{k}", [C, g["h"] * PW], f32))
        else:
            tmps.append(None)
    out_t = nc.alloc_sbuf_tensor("out_t", [C, nroi * PH * PW], f32)

    chunk_sems = [nc.alloc_semaphore(f"cs{ci}") for ci in range(nchunk)]
    gps_sem = nc.alloc_semaphore("gps_sem")
    red_sem = nc.alloc_semaphore("red_sem")
    out_sem = nc.alloc_semaphore("out_sem")

    prep_eng = _assign_prep(geoms)

    # which chunk a ROI's window is complete at
    def roi_chunk(g):
        need = g["uy2"] - ylo + 1
        for ci, (r0, r1) in enumerate(chunks):
            if r1 >= need:
                return ci
        return nchunk - 1

    roi_chunks = [roi_chunk(g) for g in geoms]

    # gps_sem schedule: 1 after the up-front memset phase, then +1 per
    # gps-copy ROI in ROI order.
    gps_count = {}
    ng = 1
    for k in range(nroi):
        if prep_eng[k][0] == "gps":
            ng += 1
        gps_count[k] = ng

    # output pieces: all-but-last-two, then the rest, for tail overlap
    n1 = max(1, nroi - 2)
    pieces = [(0, n1), (n1, nroi)] if n1 < nroi else [(0, nroi)]

    def band_ap(row0, nr, col0, ncol, inner=None):
        """AP into band: rows absolute-rel-ylo."""
        dims = [[W, nr], [1, ncol]] if inner is None else inner
        return bass.AP(band, row0 * W + col0, [[nrows * W, C]] + dims)

    def val_ap(k, off, dims):
        return bass.AP(vals[k], off, [[geoms[k]["h"] * geoms[k]["w"], C]] + dims)

    def tmp_ap(k, off, dims):
        return bass.AP(tmps[k], off, [[geoms[k]["h"] * PW, C]] + dims)

    def out_ap(off, dims):
        return bass.AP(out_t, off, [[nroi * PH * PW, C]] + dims)

    # semaphore values persist across NEFF loads on a device: clear ours
    # first; the block-exit barrier makes the clears visible to every engine
    # before any wait in the main block.
    with nc.Block() as blk0:

        @blk0.gpsimd
        def _(gpsimd):
            for s in [*chunk_sems, gps_sem, red_sem, out_sem]:
                gpsimd.sem_clear(s)

    with nc.Block() as block:

        @block.sync
        def _(sync):
            for ci, (r0, r1) in enumerate(chunks):
                sync.dma_start(
                    bass.AP(band, r0 * W, [[nrows * W, C], [1, (r1 - r0) * W]]),
                    feat_ds[ci].ap()).then_inc(chunk_sems[ci], 16)
            for pi, (p0, p1) in enumerate(pieces):
                sync.wait_ge(red_sem, pi + 1)
                d0 = p0 * PH * PW
                sync.dma_start(
                    out_d.ap()[:, d0:p1 * PH * PW],
                    out_ap(d0, [[1, (p1 - p0) * PH * PW]])).then_inc(out_sem, 16)
            sync.wait_ge(out_sem, 16 * len(pieces))

        def emit_copies(eng_obj, k, first):
            """Copy mask rects band->val; embed first[0] wait in first instr."""
            g = geoms[k]
            w = g["w"]
            inst = None
            for (r0, r1, c0, c1) in g["mrects"]:
                src = band_ap(g["uy1"] - ylo + r0, r1 - r0, g["ux1"] + c0, c1 - c0)
                dst = val_ap(k, r0 * w + c0, [[w, r1 - r0], [1, c1 - c0]])
                inst = eng_obj.tensor_copy(dst, src)
                if first[0] is not None:
                    inst._wait_ge(*first[0])
                    first[0] = None
            return inst

        @block.gpsimd
        def _(gpsimd):
            from concourse import mybir as mb
            # phase 1: every ROI's complement/tmp memsets (no data deps)
            for k in range(nroi):
                g = geoms[k]
                w = g["w"]
                for (r0, r1, c0, c1) in g["rects"]:
                    gpsimd.memset(
                        val_ap(k, r0 * w + c0, [[w, r1 - r0], [1, c1 - c0]]), 0.0)
                if not g["one_stage"]:
                    for (off, dims) in g["tmemsets"]:
                        gpsimd.memset(tmp_ap(k, off, [list(d) for d in dims]), 0.0)
            gpsimd.maybe_drain_then_inc((gps_sem, 1))
            # phase 2: copies (+ x-stage chains) for gps-assigned ROIs
            for k in range(nroi):
                if prep_eng[k][0] != "gps":
                    continue
                g = geoms[k]
                w = g["w"]
                cs = g["cs"]
                first = [(chunk_sems[roi_chunks[k]], 16)]
                emit_copies(gpsimd, k, first)
                gpsimd.maybe_drain_then_inc((gps_sem, 1))

        @block.vector
        def _(vector):
            from concourse import mybir as mb
            piece_ends = {p1 - 1: pi for pi, (p0, p1) in enumerate(pieces)}
            # all phase-1 memsets visible before any DVE work
            vector.wait_ge(gps_sem, 1)
            for k in range(nroi):
                g = geoms[k]
                h, w = g["h"], g["w"]
                rs, cs = g["rs"], g["cs"]
                if prep_eng[k][0] == "gps":
                    wait = (gps_sem, gps_count[k])
                else:
                    # copies inline on DVE; first copy waits the chunk
                    first = [(chunk_sems[roi_chunks[k]], 16)]
                    emit_copies(vector, k, first)
                    vector.drain()
                    wait = None
                first = [wait]

                def red(out, in_, axis):
                    inst = vector.tensor_reduce(out, in_, axis=axis,
                                                op=mb.AluOpType.max)
                    if first[0] is not None:
                        inst._wait_ge(*first[0])
                        first[0] = None
                    return inst

                if g["one_stage"]:
                    for (i0, ni, gi, hgt) in g["iruns"]:
                        for (j0, nj, gj, wdt) in g["jruns"]:
                            in_ap = val_ap(
                                k, int(rs[i0]) * w + int(cs[j0]),
                                [[gi * w, ni], [gj, nj], [w, hgt], [1, wdt]])
                            o_ap = out_ap(k * PH * PW + i0 * PW + j0,
                                          [[PW, ni], [1, nj]])
                            red(o_ap, in_ap, mb.AxisListType.XY)
                else:
                    if prep_eng[k][1] != "gps":
                        for (a, nrow, runs) in g["xinstrs"]:
                            for (j0, nj, gj, wdt) in runs:
                                in_ap = val_ap(k, a * w + int(cs[j0]),
                                               [[w, nrow], [gj, nj], [1, wdt]])
                                o_ap = tmp_ap(k, a * PW + j0,
                                              [[PW, nrow], [1, nj]])
                                red(o_ap, in_ap, mb.AxisListType.X)
                    vector.drain()
                    for (i0, ni, gi, hgt) in g["iruns"]:
                        in_ap = tmp_ap(k, int(rs[i0]) * PW,
                                       [[gi * PW, ni], [1, PW], [PW, hgt]])
                        o_ap = out_ap(k * PH * PW + i0 * PW, [[PW, ni], [1, PW]])
                        red(o_ap, in_ap, mb.AxisListType.X)
                if k in piece_ends:
                    vector.maybe_drain_then_inc((red_sem, 1))

    nc.compile()
    return nc


# ---------------------------------------------------------------- top level

def _prepare(feature_map, rois_1, rois_2):
    geoms = _geometry(rois_1, rois_2)
    groups = _partition_balanced(geoms)
    fm = np.ascontiguousarray(np.asarray(feature_map), np.float32)[0]
    fmb = fm.astype(ml_dtypes.bfloat16)
    programs, in_maps, core_ids = [], [], []
    for c in range(NCORES):
        ids = sorted(groups[c], key=lambda b: geoms[b]["uy2"])
        lead = min(range(min(4, len(ids))),
                   key=lambda t: geoms[ids[t]]["uy2"] - geoms[ids[t]]["uy1"])
        ids.insert(0, ids.pop(lead))
        core_geoms = [geoms[b] for b in ids]
        ylo = min(g["uy1"] for g in core_geoms)
        yhi = max(g["uy2"] for g in core_geoms) + 1
        nrows = yhi - ylo
        programs.append(_build_core_program(core_geoms, ylo, nrows))
        im = {}
        for ci, (r0, r1) in enumerate(_chunks_for(core_geoms, ylo, nrows)):
            im[f"feat{ci}"] = np.ascontiguousarray(
                fmb[:, ylo + r0:ylo + r1, :]).reshape(C, (r1 - r0) * W)
        in_maps.append(im)
        core_ids.append(ids)
    return programs, in_maps, core_ids


def _assemble(outs, core_ids):
    full = np.empty((NROIS, C, PH, PW), np.float32)
    for c in range(NCORES):
        nroi = len(core_ids[c])
        r = outs[c]["out"].reshape(C, nroi, PH, PW).transpose(1, 0, 2, 3)
        for k, b in enumerate(core_ids[c]):
            full[b] = r[k]
    return full


def _dispatch_async(nc, in_map, device):
    import jax
    from concourse import bass2jax, mybir

    bass2jax.install_neuronx_cc_hook()
    partition_name = (nc.partition_id_tensor.name
                      if nc.partition_id_tensor else None)
    in_names, out_names, out_avals, zero_outs = [], [], [], []
    for alloc in nc.m.functions[0].allocations:
        if not isinstance(alloc, mybir.MemoryLocationSet):
            continue
        name = alloc.memorylocations[0].name
        if alloc.kind == "ExternalInput":
            if name != partition_name:
                in_names.append(name)
        elif alloc.kind == "ExternalOutput":
            out_names.append(name)
            shape = tuple(alloc.tensor_shape)
            dtype = mybir.dt.np(alloc.dtype)
            out_avals.append(jax.core.ShapedArray(shape, dtype))
            zero_outs.append(np.zeros(shape, dtype))
    n_params = len(in_names)
    all_in_names = tuple(in_names + out_names
                         + ([partition_name] if partition_name else []))
    donate = tuple(range(n_params, n_params + len(out_names)))

    def _body(*args):
        operands = list(args)
        if partition_name is not None:
            operands.append(bass2jax.partition_id_tensor())
        return tuple(bass2jax._bass_exec_p.bind(
            *operands,
            out_avals=tuple(out_avals),
            in_names=all_in_names,
            out_names=tuple(out_names),
            lowering_input_output_aliases=(),
            sim_require_finite=True,
            sim_require_nnan=True,
            nc=nc,
        ))

    ins = [np.asarray(in_map[name]) for name in in_names]
    with jax.default_device(device):
        out_arrs = jax.jit(_body, donate_argnums=donate, keep_unused=True)(
            *ins, *zero_outs)
    return out_names, out_arrs


def kernel(feature_map, rois_1, rois_2):
    import jax

    programs, in_maps, core_ids = _prepare(feature_map, rois_1, rois_2)
    devices = jax.devices()
    pending = [
        _dispatch_async(programs[c], in_maps[c], devices[c])
        for c in range(NCORES)
    ]
    outs = [
        {name: np.asarray(arr) for name, arr in zip(names, arrs)}
        for names, arrs in pending
    ]
    return _assemble(outs, core_ids)
